# revision 5
# baseline (speedup 1.0000x reference)
"""2-layer GAT on 8 NeuronCores (Trainium2, Bass/Tile).

Strategy (dst-sharded graph parallel):
  - Each core owns 12500 dst nodes (padded to 12544 = 98*128, with 22 pad
    rows in EACH half so every 25088-row AllGather window has pad rows).
  - Layer 0: NO device-side gather. The host pre-gathers x rows per edge
    slot into transposed [128-feat, 128-dst] bf16 blocks; the device
    streams them sequentially (HWDGE, full bandwidth) and computes each
    slot's [feats|a_s] = x_src @ W0cat on the PE (one matmul per slot
    column, PSUM-batched 7 per bank). Pad slots are zero columns killed
    by an additive -1e30 logit mask.
  - Layer 1: node table rows [hW1|a_s1] in bf16 at 256B stride, AllGather
    (half overlap), then bulk dma_gather (custom Q7 SWDGE, int16 indices
    windowed to 25088-row segments, single_packet=False) rotating over 4
    SWDGE queues -- ~4x the descriptor-generation rate of per-column
    indirect DMAs.
  - Segment softmax without max-subtraction, weighted sum on DVE, ELU
    from min/exp/max, layer-1 projection fused per tile on the PE.
"""
import sys

sys.path.insert(0, "/opt/trn_rl_repo")

import numpy as np
import ml_dtypes

import concourse.bass as bass
import concourse.mybir as mybir
import concourse.tile as tile
from concourse import bacc
from concourse.masks import make_identity
from concourse.bass_utils import run_bass_kernel_spmd

P = 128
NCORES = 8
NEG_SLOPE = 0.2
PAD_AS = -1e30
MMB = 7          # slot-matmul columns batched per PSUM bank (7*72 <= 512 f32)
NW = 4           # layer-1 gather windows (25088 rows each, int16-addressable)
SUBC = 15        # max slot columns per dma_gather call (15*128 descs)
NQ = 4           # SWDGE queues


class Cfg:
    def __init__(self, n=100000, e=1600000, fin=128, heads=8, ch=8, out=64):
        self.N, self.E, self.IN, self.H, self.C, self.OUT = n, e, fin, heads, ch, out
        self.n_per = n // NCORES                      # owned real nodes
        self.blocks = (self.n_per + P - 1) // P       # tiles per device
        self.n_pad = self.blocks * P                  # padded nodes per device
        self.rows = NCORES * self.n_pad               # table rows
        self.hp = self.n_pad // 2                     # rows per half (6272)
        self.hreal = self.n_per // 2                  # real rows per half (6250)
        self.wrows = NCORES * self.hp // 2            # rows per gather window
        self.w0cols = out + heads + heads             # feats | a_s | a_d
        self.t0cols = out + heads                     # slot cols layer0
        self.t1b = P                                  # bf16 table row pitch


def _prep(cfg, x, edge_index):
    """Host-side sharding: permutation, per-device slot tables."""
    N, n_per, blocks, n_pad = cfg.N, cfg.n_per, cfg.blocks, cfg.n_pad
    hp, hreal, wrows = cfg.hp, cfg.hreal, cfg.wrows
    src = np.asarray(edge_index[0], dtype=np.int64)
    dst = np.asarray(edge_index[1], dtype=np.int64)
    loops = np.arange(N, dtype=np.int64)
    src = np.concatenate([src, loops])
    dst = np.concatenate([dst, loops])

    owner = dst // n_per
    deg = np.bincount(dst, minlength=N)

    perm_l = []     # perm_l[d] = original node ids in canonical rank order
    cpos_of = None  # rank -> canonical padded position (pads mid-table)
    src_cols = []   # per-device per-tile [P, K0] raw src ids (-1 pad), L0
    k0_l = []       # per-device per-tile K0 (layer-0 slot columns)
    w16_l = []      # per-device per-tile per-window [P, K1w] int16 rows
    k1_l = []       # per-device per-tile [NW] counts
    g_of = np.empty(N, dtype=np.int64)   # node -> global table row
    cpos_of = np.arange(n_per)
    cpos_of = cpos_of + (hp - hreal) * (cpos_of >= hreal)   # skip mid pads
    for d in range(NCORES):
        lo, hi = d * n_per, (d + 1) * n_per
        nodes = np.arange(lo, hi)
        order = np.argsort(-deg[lo:hi], kind="stable")
        canon = nodes[order]
        perm_l.append(canon)
        cp = cpos_of                                 # padded canonical pos
        half = cp // hp
        g_of[canon] = half * NCORES * hp + d * hp + (cp - half * hp)

    for d in range(NCORES):
        lo, hi = d * n_per, (d + 1) * n_per
        m = owner == d
        es, ed = src[m], dst[m]
        pos = np.empty(n_per, dtype=np.int64)
        pos[(perm_l[d] - lo)] = np.arange(n_per)
        ep = cpos_of[pos[ed - lo]]                   # padded canonical pos
        order = np.argsort(ep, kind="stable")
        es, ep = es[order], ep[order]
        counts = np.bincount(ep, minlength=n_pad)
        starts = np.concatenate([[0], np.cumsum(counts)])
        gsrc = g_of[es]
        wsrc = gsrc // wrows
        k0s, scols = [], []
        w16s, k1s = [], []
        for t in range(blocks):
            c = counts[t * P:(t + 1) * P]
            K0 = max(1, int(c.max()))
            k0s.append(K0)
            tsrc = np.full((P, K0), -1, dtype=np.int64)
            rows_pw = [[None] * P for _ in range(NW)]
            k1 = [1] * NW
            for p in range(P):
                node = t * P + p
                s0, s1 = starts[node], starts[node + 1]
                if s1 > s0:
                    tsrc[p, :s1 - s0] = es[s0:s1]
                    for w in range(NW):
                        sel = wsrc[s0:s1] == w
                        if sel.any():
                            r = gsrc[s0:s1][sel] - w * wrows
                            rows_pw[w][p] = r.astype(np.int16)
                            k1[w] = max(k1[w], len(r))
            scols.append(tsrc)
            tw = []
            for w in range(NW):
                buf = np.full((P, k1[w]), hreal, dtype=np.int16)  # pad row
                for p in range(P):
                    r = rows_pw[w][p]
                    if r is not None:
                        buf[p, :len(r)] = r
                tw.append(buf)
            w16s.append(tw)
            k1s.append(k1)
        src_cols.append(scols)
        k0_l.append(k0s)
        w16_l.append(w16s)
        k1_l.append(k1s)
    return perm_l, cpos_of, src_cols, k0_l, w16_l, k1_l


def _subcalls(k):
    """Split k columns into chunks of <= SUBC."""
    out = []
    o = 0
    while o < k:
        c = min(SUBC, k - o)
        out.append(c)
        o += c
    return out


def _build(cfg, kmax0, ncols0, k1u, nidx16):
    H, C, OUT = cfg.H, cfg.C, cfg.OUT
    n_pad, blocks, rows = cfg.n_pad, cfg.blocks, cfg.rows
    T0, T1B = cfg.t0cols, cfg.t1b
    hp, hreal, wrows = cfg.hp, cfg.hreal, cfg.wrows
    f32 = mybir.dt.float32
    bf16 = mybir.dt.bfloat16
    i16 = mybir.dt.int16

    nc = bacc.Bacc(num_devices=NCORES, num_swdge_queues=NQ)
    xt = nc.declare_dram_parameter("xt", [cfg.IN, n_pad], bf16, isOutput=False)
    xg = nc.declare_dram_parameter("xg", [P, ncols0 * P], bf16, isOutput=False)
    msk = nc.declare_dram_parameter("msk", [P, ncols0], f32, isOutput=False)
    ix16 = nc.declare_dram_parameter("ix16", [P, nidx16], i16, isOutput=False)
    w0 = nc.declare_dram_parameter("w0", [cfg.IN, cfg.w0cols], bf16, isOutput=False)
    w1 = nc.declare_dram_parameter("w1", [P, cfg.OUT + 2], f32, isOutput=False)
    bias = nc.declare_dram_parameter("bias", [2, cfg.OUT], f32, isOutput=False)
    out_d = nc.declare_dram_parameter("out", [n_pad, cfg.OUT], f32, isOutput=True)

    ltab1 = nc.dram_tensor("ltab1", [n_pad, T1B], bf16)
    tab1 = nc.dram_tensor("tab1", [rows, T1B], bf16, addr_space="Shared")

    with tile.TileContext(nc) as tc:
        with (
            tc.tile_pool(name="persist", bufs=1) as pp,
            tc.tile_pool(name="work", bufs=3) as wp,
            tc.tile_pool(name="gat", bufs=2) as gp,
            tc.tile_pool(name="ps", bufs=2, space="PSUM") as psp,
            tc.tile_pool(name="ps_slot", bufs=2, space="PSUM") as pss,
        ):
            # ---- constants ----
            w0t = pp.tile([cfg.IN, cfg.w0cols], bf16)
            nc.sync.dma_start(out=w0t[:], in_=w0[:])
            w1t = pp.tile([P, cfg.OUT + 2], f32)
            nc.sync.dma_start(out=w1t[:], in_=w1[:])
            b0t = pp.tile([P, cfg.OUT], f32)
            nc.sync.dma_start(out=b0t[:], in_=bias[0:1, :].to_broadcast([P, cfg.OUT]))
            b1t = pp.tile([P, cfg.OUT], f32)
            nc.sync.dma_start(out=b1t[:], in_=bias[1:2, :].to_broadcast([P, cfg.OUT]))
            mskt = pp.tile([P, ncols0], f32)
            nc.sync.dma_start(out=mskt[:], in_=msk[:])
            a_d0 = pp.tile([P, blocks * H], f32)
            a_d1 = pp.tile([P, blocks], f32)
            ident = pp.tile([P, P], f32)
            make_identity(nc, ident[:])
            pad_as = pp.tile([P, H], bf16)
            nc.vector.memset(pad_as[:], PAD_AS)

            half_t = hp // P

            # ---- P1: a_d0 per node (tiny matmuls) ----
            for t in range(blocks):
                xs = wp.tile([cfg.IN, P], bf16, tag="xs")
                nc.sync.dma_start(out=xs[:], in_=xt[:, t * P:(t + 1) * P])
                ps = psp.tile([P, H], f32, tag="mm0")
                nc.tensor.matmul(out=ps[:], lhsT=xs[:],
                                 rhs=w0t[:, T0:T0 + H],
                                 start=True, stop=True)
                nc.vector.tensor_copy(out=a_d0[:, t * H:(t + 1) * H], in_=ps[:])

            # ---- L0 edge phase (host-pregathered slots) + fused L1 proj ----
            col = 0
            for t in range(blocks):
                K = kmax0[t]
                g = gp.tile([P, K * T0], f32, tag="g0")
                for k0 in range(0, K, MMB):
                    cnt = min(MMB, K - k0)
                    xgc = wp.tile([P, cnt * P], bf16, tag="xgc")
                    nc.sync.dma_start(
                        out=xgc[:],
                        in_=xg[:, (col + k0) * P:(col + k0 + cnt) * P])
                    ps = pss.tile([P, cnt * T0], f32, tag="slotmm")
                    for j in range(cnt):
                        nc.tensor.matmul(
                            out=ps[:, j * T0:(j + 1) * T0],
                            lhsT=xgc[:, j * P:(j + 1) * P],
                            rhs=w0t[:, 0:T0], start=True, stop=True)
                    nc.scalar.copy(out=g[:, k0 * T0:(k0 + cnt) * T0], in_=ps[:])
                gv = g[:].rearrange("p (k w) -> p k w", w=T0)
                e = gp.tile([P, H * K], f32, tag="e")
                ev = e[:].rearrange("p (h k) -> p h k", k=K)
                asg = gv[:, :, cfg.OUT:T0].rearrange("p k h -> p h k")
                nc.vector.tensor_tensor(
                    out=ev, in0=asg,
                    in1=a_d0[:, t * H:(t + 1) * H].to_broadcast([P, H, K]),
                    op=mybir.AluOpType.add)
                mv = mskt[:, col:col + K]
                mb = bass.AP(mv.tensor, mv.offset,
                             [mv.ap[0], [0, H], mv.ap[1]])
                nc.vector.tensor_tensor(out=ev, in0=ev, in1=mb,
                                        op=mybir.AluOpType.add)
                col += K
                scr = gp.tile([P, H * K], f32, tag="scr")
                nc.vector.tensor_scalar(
                    out=scr[:], in0=e[:], scalar1=NEG_SLOPE, scalar2=-88.0,
                    op0=mybir.AluOpType.mult, op1=mybir.AluOpType.max)
                nc.vector.tensor_tensor(out=e[:], in0=e[:], in1=scr[:],
                                        op=mybir.AluOpType.max)
                nc.scalar.activation(out=e[:], in_=e[:],
                                     func=mybir.ActivationFunctionType.Exp)
                den = gp.tile([P, H], f32, tag="den")
                nc.vector.tensor_reduce(out=den[:], in_=ev,
                                        axis=mybir.AxisListType.X,
                                        op=mybir.AluOpType.add)
                nc.vector.reciprocal(out=den[:], in_=den[:])
                nc.vector.tensor_tensor(
                    out=ev, in0=ev,
                    in1=den[:].to_broadcast([P, H, K]),
                    op=mybir.AluOpType.mult)
                prod = gp.tile([P, cfg.OUT * K], f32, tag="prod")
                pv = prod[:].rearrange("p (h c k) -> p h c k", c=C, k=K)
                al_b = bass.AP(ev.tensor, ev.offset,
                               [ev.ap[0], ev.ap[1], [0, C], ev.ap[2]])
                nc.vector.tensor_tensor(
                    out=pv,
                    in0=al_b,
                    in1=gv[:, :, 0:cfg.OUT].rearrange(
                        "p k (h c) -> p h c k", c=C),
                    op=mybir.AluOpType.mult)
                hfeat = gp.tile([P, cfg.OUT], f32, tag="hfeat")
                nc.vector.tensor_reduce(
                    out=hfeat[:], in_=pv, axis=mybir.AxisListType.X,
                    op=mybir.AluOpType.add)
                nc.vector.tensor_add(out=hfeat[:], in0=hfeat[:], in1=b0t[:])
                # ELU: h = max(x,0) + exp(min(x,0)) - 1
                tmn = gp.tile([P, cfg.OUT], f32, tag="tmn")
                nc.vector.tensor_scalar_min(out=tmn[:], in0=hfeat[:], scalar1=0.0)
                nc.scalar.activation(out=tmn[:], in_=tmn[:],
                                     func=mybir.ActivationFunctionType.Exp)
                nc.vector.tensor_scalar_max(out=hfeat[:], in0=hfeat[:], scalar1=0.0)
                nc.vector.tensor_tensor(out=hfeat[:], in0=hfeat[:], in1=tmn[:],
                                        op=mybir.AluOpType.add)
                nc.vector.tensor_scalar_add(out=hfeat[:], in0=hfeat[:], scalar1=-1.0)
                # L1 projection: rows of ltab1 = [h @ W1 | h @ w_src1]
                pst = psp.tile([P, P], f32, tag="tr")
                nc.tensor.transpose(out=pst[:cfg.OUT, :], in_=hfeat[:],
                                    identity=ident[:])
                ht = wp.tile([cfg.OUT, P], f32, tag="ht")
                nc.scalar.copy(out=ht[:], in_=pst[:cfg.OUT, :])
                ps1 = psp.tile([P, cfg.OUT + 2], f32, tag="mm1")
                nc.tensor.matmul(out=ps1[:], lhsT=ht[:],
                                 rhs=w1t[:cfg.OUT, :], start=True, stop=True)
                row1 = wp.tile([P, cfg.OUT + 1], bf16, tag="row1")
                nc.scalar.copy(out=row1[:], in_=ps1[:, 0:cfg.OUT + 1])
                nc.vector.tensor_copy(out=a_d1[:, t:t + 1],
                                      in_=ps1[:, cfg.OUT + 1:cfg.OUT + 2])
                nc.sync.dma_start(out=ltab1[t * P:(t + 1) * P, 0:cfg.OUT + 1],
                                  in_=row1[:])
                if t == half_t - 1:
                    # pad rows of half 0: a_s = -1e30, then gather half 0
                    nc.sync.dma_start(
                        out=ltab1[hreal:hp, cfg.OUT:cfg.OUT + 1],
                        in_=pad_as[0:hp - hreal, 0:1])
                    nc.gpsimd.collective_compute(
                        "AllGather", mybir.AluOpType.bypass,
                        replica_groups=[list(range(NCORES))],
                        ins=[ltab1[0:hp, :]],
                        outs=[tab1[0:NCORES * hp, :]])
            nc.sync.dma_start(
                out=ltab1[hp + hreal:n_pad, cfg.OUT:cfg.OUT + 1],
                in_=pad_as[0:hp - hreal, 0:1])
            nc.gpsimd.collective_compute(
                "AllGather", mybir.AluOpType.bypass,
                replica_groups=[list(range(NCORES))],
                ins=[ltab1[hp:n_pad, :]],
                outs=[tab1[NCORES * hp:rows, :]])

            # ---- L1 edge phase (bulk dma_gather over 4 windows/queues) ----
            icol = 0      # int16 column offset into ix16
            qrr = 0       # queue round-robin
            for t in range(blocks):
                k1 = k1u[t]
                K = sum(k1)
                g = gp.tile([P, K * T1B], bf16, tag="g1")
                gq = g[:].rearrange("p (k w) -> p k w", w=T1B)
                ixlen = K * 8
                ixt = wp.tile([P, ixlen], i16, tag="ix")
                nc.sync.dma_start(out=ixt[:], in_=ix16[:, icol:icol + ixlen])
                icol += ixlen
                co = 0    # column offset within this tile
                ico = 0   # int16 col offset within ixt
                for w in range(NW):
                    for cnt in _subcalls(k1[w]):
                        nc.gpsimd.dma_gather(
                            out_ap=gq[:, co:co + cnt, :],
                            in_ap=tab1[w * wrows:(w + 1) * wrows, :],
                            idxs_ap=ixt[:, ico:ico + cnt * 8],
                            num_idxs=cnt * P,
                            num_idxs_reg=cnt * P,
                            elem_size=T1B,
                            single_packet=False,
                            queue_num=qrr % NQ)
                        qrr += 1
                        co += cnt
                        ico += cnt * 8
                e = gp.tile([P, K], f32, tag="e1")
                asg1 = gq[:, :, cfg.OUT:cfg.OUT + 1].rearrange("p k w -> p (k w)")
                nc.vector.tensor_tensor(
                    out=e[:], in0=asg1,
                    in1=a_d1[:, t:t + 1].to_broadcast([P, K]),
                    op=mybir.AluOpType.add)
                scr1 = gp.tile([P, K], f32, tag="scr1")
                nc.vector.tensor_scalar(
                    out=scr1[:], in0=e[:], scalar1=NEG_SLOPE, scalar2=-88.0,
                    op0=mybir.AluOpType.mult, op1=mybir.AluOpType.max)
                nc.vector.tensor_tensor(out=e[:], in0=e[:], in1=scr1[:],
                                        op=mybir.AluOpType.max)
                nc.scalar.activation(out=e[:], in_=e[:],
                                     func=mybir.ActivationFunctionType.Exp)
                den = gp.tile([P, 1], f32, tag="den1")
                nc.vector.tensor_reduce(out=den[:], in_=e[:],
                                        axis=mybir.AxisListType.X,
                                        op=mybir.AluOpType.add)
                nc.vector.reciprocal(out=den[:], in_=den[:])
                nc.vector.tensor_tensor(
                    out=e[:], in0=e[:], in1=den[:].to_broadcast([P, K]),
                    op=mybir.AluOpType.mult)
                prod = gp.tile([P, cfg.OUT * K], f32, tag="prod1")
                pv = prod[:].rearrange("p (c k) -> p c k", k=K)
                e_ap = e[:]
                al_b = bass.AP(e_ap.tensor, e_ap.offset,
                               [e_ap.ap[0], [0, cfg.OUT], e_ap.ap[1]])
                nc.vector.tensor_tensor(
                    out=pv,
                    in0=al_b,
                    in1=gq[:, :, 0:cfg.OUT].rearrange("p k c -> p c k"),
                    op=mybir.AluOpType.mult)
                of = gp.tile([P, cfg.OUT], f32, tag="of")
                nc.vector.tensor_reduce(out=of[:], in_=pv,
                                        axis=mybir.AxisListType.X,
                                        op=mybir.AluOpType.add)
                nc.vector.tensor_add(out=of[:], in0=of[:], in1=b1t[:])
                tmn = gp.tile([P, cfg.OUT], f32, tag="tmn1")
                nc.vector.tensor_scalar_min(out=tmn[:], in0=of[:], scalar1=0.0)
                nc.scalar.activation(out=tmn[:], in_=tmn[:],
                                     func=mybir.ActivationFunctionType.Exp)
                nc.vector.tensor_scalar_max(out=of[:], in0=of[:], scalar1=0.0)
                nc.vector.tensor_tensor(out=of[:], in0=of[:], in1=tmn[:],
                                        op=mybir.AluOpType.add)
                nc.vector.tensor_scalar_add(out=of[:], in0=of[:], scalar1=-1.0)
                nc.sync.dma_start(out=out_d[t * P:(t + 1) * P, :], in_=of[:])
    nc.finalize()
    return nc


def kernel(x, edge_index, W0, att_src0, att_dst0, b0, W1, att_src1, att_dst1, b1,
           _cfg=None):
    cfg = _cfg or Cfg()
    x = np.asarray(x, dtype=np.float32)
    W0 = np.asarray(W0, np.float32)
    W1 = np.asarray(W1, np.float32)
    att_src0 = np.asarray(att_src0, np.float32)
    att_dst0 = np.asarray(att_dst0, np.float32)
    att_src1 = np.asarray(att_src1, np.float32)
    att_dst1 = np.asarray(att_dst1, np.float32)
    b0 = np.asarray(b0, np.float32)
    b1 = np.asarray(b1, np.float32)

    perm_l, cpos_of, src_cols, k0_l, w16_l, k1_l = _prep(cfg, x, edge_index)
    blocks = cfg.blocks
    # unify per-tile K across devices (SPMD: one program)
    kmax0 = [max(k0_l[d][t] for d in range(NCORES)) for t in range(blocks)]
    ncols0 = int(np.sum(kmax0))
    k1u = [[max(k1_l[d][t][w] for d in range(NCORES)) for w in range(NW)]
           for t in range(blocks)]
    nidx16 = int(sum(sum(k1u[t]) for t in range(blocks))) * 8

    x_bf = x.astype(ml_dtypes.bfloat16)
    xz = np.zeros((1, cfg.IN), ml_dtypes.bfloat16)
    x_ext = np.concatenate([x_bf, xz])     # row N = zeros for pad slots
    xg_u, msk_u, ix_u = [], [], []
    for d in range(NCORES):
        sbuf_ = np.full((P, ncols0), cfg.N, dtype=np.int64)
        c_d = 0
        for t in range(blocks):
            kd = k0_l[d][t]
            s = src_cols[d][t]
            sbuf_[:, c_d:c_d + kd] = np.where(s < 0, cfg.N, s)
            c_d += kmax0[t]
        msk_u.append(np.where(sbuf_ == cfg.N, PAD_AS, 0.0).astype(np.float32))
        xgd = x_ext[sbuf_.T.reshape(-1)]           # [ncols0*128, IN] bf16
        xgd = xgd.reshape(ncols0, P, cfg.IN).transpose(0, 2, 1)
        xg_u.append(np.ascontiguousarray(
            xgd.transpose(1, 0, 2).reshape(cfg.IN, ncols0 * P)))
        # layer-1 int16 index stream: per tile, per window, per subcall,
        # wrapped [16, cnt*8] blocks replicated to 128 partitions
        parts = []
        for t in range(blocks):
            for w in range(NW):
                have = k1_l[d][t][w]
                buf = np.full((P, k1u[t][w]), cfg.hreal, dtype=np.int16)
                buf[:, :have] = w16_l[d][t][w]
                o = 0
                for cnt in _subcalls(k1u[t][w]):
                    cb = buf[:, o:o + cnt]           # [128, cnt]
                    logical = cb.T.reshape(-1)       # k-major: k*128+p
                    wrapped = logical.reshape(-1, 16).T   # [16, cnt*8]
                    parts.append(np.tile(wrapped, (8, 1)))
                    o += cnt
        ix_u.append(np.concatenate(parts, axis=1))
        assert ix_u[-1].shape == (P, nidx16), ix_u[-1].shape

    # weights: A blockdiag for layer0 attention
    H, C = cfg.H, cfg.C
    A_src = np.zeros((H * C, H), np.float32)
    A_dst = np.zeros((H * C, H), np.float32)
    for h in range(H):
        A_src[h * C:(h + 1) * C, h] = att_src0[h]
        A_dst[h * C:(h + 1) * C, h] = att_dst0[h]
    w0cat = np.concatenate([W0, W0 @ A_src, W0 @ A_dst], axis=1)  # [IN, 80]
    w1cat = np.zeros((P, cfg.OUT + 2), np.float32)
    w1cat[:cfg.OUT] = np.concatenate(
        [W1, W1 @ att_src1[0][:, None], W1 @ att_dst1[0][:, None]], axis=1)
    biases = np.stack([b0.reshape(-1), b1.reshape(-1)])

    nc = _build(cfg, kmax0, ncols0, k1u, nidx16)
    in_maps = []
    for d in range(NCORES):
        xt = np.zeros((cfg.IN, cfg.n_pad), ml_dtypes.bfloat16)
        xt[:, cpos_of] = x_bf[perm_l[d]].T
        in_maps.append({
            "xt": xt,
            "xg": xg_u[d],
            "msk": msk_u[d],
            "ix16": ix_u[d],
            "w0": w0cat.astype(ml_dtypes.bfloat16),
            "w1": w1cat,
            "bias": biases,
        })
    res = run_bass_kernel_spmd(nc, in_maps, core_ids=list(range(NCORES)))
    out = np.empty((cfg.N, cfg.OUT), np.float32)
    for d in range(NCORES):
        out[perm_l[d]] = res.results[d]["out"][cpos_of]
    return out


# revision 6
# speedup vs baseline: 1.2817x; 1.2817x over previous
"""2-layer GAT on 8 NeuronCores (Trainium2, Bass/Tile).

Strategy (dst-sharded graph parallel):
  - Each core owns 12500 dst nodes (padded to 12544 = 98*128).
  - Layer 0: NO device-side gather. The host pre-gathers x rows per edge
    slot into transposed [128-feat, 128-dst] bf16 blocks; the device
    streams them sequentially (HWDGE, full bandwidth) and computes each
    slot's [feats|a_s] = x_src @ W0cat directly on the PE (one matmul per
    slot column, PSUM-batched 7 columns per bank). Pad slots are zero
    columns killed by an additive -1e30 logit mask.
  - Layer 1: node table rows [hW1|a_s1] (f32), AllGather (half overlap),
    per-slot-column indirect DMA gather (SWDGE), same softmax pipeline.
  - Segment softmax without max-subtraction (logits are O(10), exact
    same alpha ratios), weighted sum on DVE, ELU composed from
    min/exp/max, layer-1 projection fused per tile on the PE.
"""
import sys

sys.path.insert(0, "/opt/trn_rl_repo")

import numpy as np
import ml_dtypes

import concourse.bass as bass
import concourse.mybir as mybir
import concourse.tile as tile
from concourse import bacc
from concourse.masks import make_identity
from concourse.bass_utils import run_bass_kernel_spmd

P = 128
NCORES = 8
NEG_SLOPE = 0.2
PAD_AS = -1e30
MMB = 7          # slot-matmul columns batched per PSUM bank (7*72 <= 512 f32)


class Cfg:
    def __init__(self, n=100000, e=1600000, fin=128, heads=8, ch=8, out=64):
        self.N, self.E, self.IN, self.H, self.C, self.OUT = n, e, fin, heads, ch, out
        self.n_per = n // NCORES                      # owned real nodes
        self.blocks = (self.n_per + P - 1) // P       # tiles per device
        self.n_pad = self.blocks * P                  # padded nodes per device
        self.rows = NCORES * self.n_pad               # table rows
        self.w0cols = out + heads + heads             # feats | a_s | a_d
        self.t0cols = out + heads                     # slot cols layer0
        self.t1cols = out + 1                         # table row cols layer1


def _prep(cfg, x, edge_index):
    """Host-side sharding: permutation, per-device tiles, gather indices."""
    N, n_per, blocks, n_pad = cfg.N, cfg.n_per, cfg.blocks, cfg.n_pad
    src = np.asarray(edge_index[0], dtype=np.int64)
    dst = np.asarray(edge_index[1], dtype=np.int64)
    loops = np.arange(N, dtype=np.int64)
    src = np.concatenate([src, loops])
    dst = np.concatenate([dst, loops])

    owner = dst // n_per
    deg = np.bincount(dst, minlength=N)

    perm_l = []          # perm_l[d] = original node ids in canonical order
    idx_cols = []        # per-device int32 [P, C] gather indices (layer 1)
    src_cols = []        # per-device int64 [P, C] raw src node ids (-1 = pad)
    kmax_l = []          # per-device list of K per tile
    g_of = np.empty(N, dtype=np.int64)   # original node -> global table row
    for d in range(NCORES):
        lo, hi = d * n_per, (d + 1) * n_per
        nodes = np.arange(lo, hi)
        order = np.argsort(-deg[lo:hi], kind="stable")
        canon = nodes[order]                       # canonical order, len n_per
        perm_l.append(canon)
        cpos = np.arange(n_per)
        hp = n_pad // 2
        g_of[canon] = (cpos // hp) * NCORES * hp + d * hp + (cpos % hp)

    # per-device edge slots
    for d in range(NCORES):
        lo, hi = d * n_per, (d + 1) * n_per
        m = owner == d
        es, ed = src[m], dst[m]
        pos = np.empty(n_per, dtype=np.int64)
        pos[(perm_l[d] - lo)] = np.arange(n_per)
        ep = pos[ed - lo]                          # canonical pos of each edge's dst
        order = np.argsort(ep, kind="stable")
        es, ep = es[order], ep[order]
        counts = np.bincount(ep, minlength=n_pad)
        starts = np.concatenate([[0], np.cumsum(counts)])
        kmax = []
        cols = []
        scols = []
        hp2 = n_pad // 2                           # device-0 pad row (a_s=-1e30)
        dummy = (cfg.n_per // hp2) * NCORES * hp2 + 0 * hp2 + (cfg.n_per % hp2)
        for t in range(blocks):
            c = counts[t * P:(t + 1) * P]
            K = max(1, int(c.max()))
            kmax.append(K)
            tilecols = np.full((P, K), dummy, dtype=np.int32)
            tsrc = np.full((P, K), -1, dtype=np.int64)
            for p in range(P):
                node = t * P + p
                s0, s1 = starts[node], starts[node + 1]
                if s1 > s0:
                    tilecols[p, :s1 - s0] = g_of[es[s0:s1]]
                    tsrc[p, :s1 - s0] = es[s0:s1]
            cols.append(tilecols)
            scols.append(tsrc)
        idx_cols.append(np.concatenate(cols, axis=1))  # [P, sum K]
        src_cols.append(np.concatenate(scols, axis=1))
        kmax_l.append(kmax)
    return perm_l, idx_cols, src_cols, kmax_l


def _build(cfg, kmax, ncols):
    H, C, OUT = cfg.H, cfg.C, cfg.OUT
    n_pad, blocks, rows = cfg.n_pad, cfg.blocks, cfg.rows
    T0, T1 = cfg.t0cols, cfg.t1cols
    f32 = mybir.dt.float32
    bf16 = mybir.dt.bfloat16

    nc = bacc.Bacc(num_devices=NCORES)
    xt = nc.declare_dram_parameter("xt", [cfg.IN, n_pad], bf16, isOutput=False)
    xg = nc.declare_dram_parameter("xg", [P, ncols * P], bf16, isOutput=False)
    msk = nc.declare_dram_parameter("msk", [P, ncols], f32, isOutput=False)
    idx = nc.declare_dram_parameter("idx", [P, ncols], mybir.dt.int32, isOutput=False)
    w0 = nc.declare_dram_parameter("w0", [cfg.IN, cfg.w0cols], bf16, isOutput=False)
    w1 = nc.declare_dram_parameter("w1", [P, cfg.OUT + 2], f32, isOutput=False)
    bias = nc.declare_dram_parameter("bias", [2, cfg.OUT], f32, isOutput=False)
    out_d = nc.declare_dram_parameter("out", [n_pad, cfg.OUT], f32, isOutput=True)

    ltab1 = nc.dram_tensor("ltab1", [n_pad, T1], f32)
    tab1 = nc.dram_tensor("tab1", [rows, T1], f32, addr_space="Shared")

    with tile.TileContext(nc) as tc:
        with (
            tc.tile_pool(name="persist", bufs=1) as pp,
            tc.tile_pool(name="work", bufs=3) as wp,
            tc.tile_pool(name="gat", bufs=2) as gp,
            tc.tile_pool(name="g1p", bufs=3) as g1p,
            tc.tile_pool(name="ps", bufs=2, space="PSUM") as psp,
            tc.tile_pool(name="ps_slot", bufs=2, space="PSUM") as pss,
        ):
            # ---- constants ----
            w0t = pp.tile([cfg.IN, cfg.w0cols], bf16)
            nc.sync.dma_start(out=w0t[:], in_=w0[:])
            w1t = pp.tile([P, cfg.OUT + 2], f32)
            nc.sync.dma_start(out=w1t[:], in_=w1[:])
            b0t = pp.tile([P, cfg.OUT], f32)
            nc.sync.dma_start(out=b0t[:], in_=bias[0:1, :].to_broadcast([P, cfg.OUT]))
            b1t = pp.tile([P, cfg.OUT], f32)
            nc.sync.dma_start(out=b1t[:], in_=bias[1:2, :].to_broadcast([P, cfg.OUT]))
            idxt = pp.tile([P, ncols], mybir.dt.int32)
            nc.sync.dma_start(out=idxt[:], in_=idx[:])
            mskt = pp.tile([P, ncols], f32)
            nc.sync.dma_start(out=mskt[:], in_=msk[:])
            a_d0 = pp.tile([P, blocks * H], f32)
            a_d1 = pp.tile([P, blocks], f32)
            ident = pp.tile([P, P], f32)
            make_identity(nc, ident[:])
            pad_as = pp.tile([P, H], f32)
            nc.vector.memset(pad_as[:], PAD_AS)

            hp = n_pad // 2
            half_t = hp // P

            # ---- P1: a_d0 per node (tiny matmuls) ----
            for t in range(blocks):
                xs = wp.tile([cfg.IN, P], bf16, tag="xs")
                nc.sync.dma_start(out=xs[:], in_=xt[:, t * P:(t + 1) * P])
                ps = psp.tile([P, H], f32, tag="mm0")
                nc.tensor.matmul(out=ps[:], lhsT=xs[:],
                                 rhs=w0t[:, T0:T0 + H],
                                 start=True, stop=True)
                nc.vector.tensor_copy(out=a_d0[:, t * H:(t + 1) * H], in_=ps[:])

            # ---- L0 edge phase (host-pregathered slots) + fused L1 proj ----
            col = 0
            for t in range(blocks):
                K = kmax[t]
                g = gp.tile([P, K * T0], f32, tag="g0")
                for k0 in range(0, K, MMB):
                    cnt = min(MMB, K - k0)
                    xgc = wp.tile([P, cnt * P], bf16, tag="xgc")
                    nc.sync.dma_start(
                        out=xgc[:],
                        in_=xg[:, (col + k0) * P:(col + k0 + cnt) * P])
                    ps = pss.tile([P, cnt * T0], f32, tag="slotmm")
                    for j in range(cnt):
                        nc.tensor.matmul(
                            out=ps[:, j * T0:(j + 1) * T0],
                            lhsT=xgc[:, j * P:(j + 1) * P],
                            rhs=w0t[:, 0:T0], start=True, stop=True)
                    nc.scalar.copy(out=g[:, k0 * T0:(k0 + cnt) * T0], in_=ps[:])
                gv = g[:].rearrange("p (k w) -> p k w", w=T0)
                # e[p,h,k] = a_s[src] + a_d[dst] + mask
                e = gp.tile([P, H * K], f32, tag="e")
                ev = e[:].rearrange("p (h k) -> p h k", k=K)
                asg = gv[:, :, cfg.OUT:T0].rearrange("p k h -> p h k")
                nc.vector.tensor_tensor(
                    out=ev, in0=asg,
                    in1=a_d0[:, t * H:(t + 1) * H].to_broadcast([P, H, K]),
                    op=mybir.AluOpType.add)
                mv = mskt[:, col:col + K]
                mb = bass.AP(mv.tensor, mv.offset,
                             [mv.ap[0], [0, H], mv.ap[1]])
                nc.vector.tensor_tensor(out=ev, in0=ev, in1=mb,
                                        op=mybir.AluOpType.add)
                col += K
                scr = gp.tile([P, H * K], f32, tag="scr")
                nc.vector.tensor_scalar(
                    out=scr[:], in0=e[:], scalar1=NEG_SLOPE, scalar2=-88.0,
                    op0=mybir.AluOpType.mult, op1=mybir.AluOpType.max)
                nc.vector.tensor_tensor(out=e[:], in0=e[:], in1=scr[:],
                                        op=mybir.AluOpType.max)
                nc.scalar.activation(out=e[:], in_=e[:],
                                     func=mybir.ActivationFunctionType.Exp)
                den = gp.tile([P, H], f32, tag="den")
                nc.vector.tensor_reduce(out=den[:], in_=ev,
                                        axis=mybir.AxisListType.X,
                                        op=mybir.AluOpType.add)
                nc.vector.reciprocal(out=den[:], in_=den[:])
                nc.vector.tensor_tensor(
                    out=ev, in0=ev,
                    in1=den[:].to_broadcast([P, H, K]),
                    op=mybir.AluOpType.mult)
                prod = gp.tile([P, cfg.OUT * K], f32, tag="prod")
                pv = prod[:].rearrange("p (h c k) -> p h c k", c=C, k=K)
                al_b = bass.AP(ev.tensor, ev.offset,
                               [ev.ap[0], ev.ap[1], [0, C], ev.ap[2]])
                nc.vector.tensor_tensor(
                    out=pv,
                    in0=al_b,
                    in1=gv[:, :, 0:cfg.OUT].rearrange(
                        "p k (h c) -> p h c k", c=C),
                    op=mybir.AluOpType.mult)
                hfeat = gp.tile([P, cfg.OUT], f32, tag="hfeat")
                nc.vector.tensor_reduce(
                    out=hfeat[:], in_=pv, axis=mybir.AxisListType.X,
                    op=mybir.AluOpType.add)
                nc.vector.tensor_add(out=hfeat[:], in0=hfeat[:], in1=b0t[:])
                # ELU: h = max(x,0) + exp(min(x,0)) - 1
                tmn = gp.tile([P, cfg.OUT], f32, tag="tmn")
                nc.vector.tensor_scalar_min(out=tmn[:], in0=hfeat[:], scalar1=0.0)
                nc.scalar.activation(out=tmn[:], in_=tmn[:],
                                     func=mybir.ActivationFunctionType.Exp)
                nc.vector.tensor_scalar_max(out=hfeat[:], in0=hfeat[:], scalar1=0.0)
                nc.vector.tensor_tensor(out=hfeat[:], in0=hfeat[:], in1=tmn[:],
                                        op=mybir.AluOpType.add)
                nc.vector.tensor_scalar_add(out=hfeat[:], in0=hfeat[:], scalar1=-1.0)
                # L1 projection: rows of ltab1 = [h @ W1 | h @ w_src1]; a_d1 kept
                pst = psp.tile([P, P], f32, tag="tr")
                nc.tensor.transpose(out=pst[:cfg.OUT, :], in_=hfeat[:],
                                    identity=ident[:])
                ht = wp.tile([cfg.OUT, P], f32, tag="ht")
                nc.scalar.copy(out=ht[:], in_=pst[:cfg.OUT, :])
                ps1 = psp.tile([P, cfg.OUT + 2], f32, tag="mm1")
                nc.tensor.matmul(out=ps1[:], lhsT=ht[:],
                                 rhs=w1t[:cfg.OUT, :], start=True, stop=True)
                row1 = wp.tile([P, T1], f32, tag="row1")
                nc.scalar.copy(out=row1[:], in_=ps1[:, 0:T1])
                nc.vector.tensor_copy(out=a_d1[:, t:t + 1],
                                      in_=ps1[:, T1:T1 + 1])
                nc.sync.dma_start(out=ltab1[t * P:(t + 1) * P, :], in_=row1[:])
                if t == half_t - 1:
                    nc.gpsimd.collective_compute(
                        "AllGather", mybir.AluOpType.bypass,
                        replica_groups=[list(range(NCORES))],
                        ins=[ltab1[0:hp, :]],
                        outs=[tab1[0:NCORES * hp, :]])
            npad_rows = n_pad - cfg.n_per
            if npad_rows > 0:
                nc.sync.dma_start(
                    out=ltab1[cfg.n_per:n_pad, cfg.OUT:cfg.OUT + 1],
                    in_=pad_as[0:npad_rows, 0:1])

            # ---- AllGather layer-1 table (2nd half) ----
            nc.gpsimd.collective_compute(
                "AllGather", mybir.AluOpType.bypass,
                replica_groups=[list(range(NCORES))],
                ins=[ltab1[hp:n_pad, :]],
                outs=[tab1[NCORES * hp:rows, :]])

            # ---- L1 edge phase ----
            col = 0
            for t in range(blocks):
                K = kmax[t]
                g = g1p.tile([P, K * T1], f32, tag="g1")
                for k in range(K):
                    nc.gpsimd.indirect_dma_start(
                        out=g[:, k * T1:(k + 1) * T1],
                        out_offset=None,
                        in_=tab1[:, :],
                        in_offset=bass.IndirectOffsetOnAxis(
                            ap=idxt[:, col + k:col + k + 1], axis=0))
                col += K
                gv = g[:].rearrange("p (k w) -> p k w", w=T1)
                e = gp.tile([P, K], f32, tag="e1")
                asg1 = gv[:, :, cfg.OUT:T1].rearrange("p k w -> p (k w)")
                nc.vector.tensor_tensor(
                    out=e[:], in0=asg1,
                    in1=a_d1[:, t:t + 1].to_broadcast([P, K]),
                    op=mybir.AluOpType.add)
                scr1 = gp.tile([P, K], f32, tag="scr1")
                nc.vector.tensor_scalar(
                    out=scr1[:], in0=e[:], scalar1=NEG_SLOPE, scalar2=-88.0,
                    op0=mybir.AluOpType.mult, op1=mybir.AluOpType.max)
                nc.vector.tensor_tensor(out=e[:], in0=e[:], in1=scr1[:],
                                        op=mybir.AluOpType.max)
                nc.scalar.activation(out=e[:], in_=e[:],
                                     func=mybir.ActivationFunctionType.Exp)
                den = gp.tile([P, 1], f32, tag="den1")
                nc.vector.tensor_reduce(out=den[:], in_=e[:],
                                        axis=mybir.AxisListType.X,
                                        op=mybir.AluOpType.add)
                nc.vector.reciprocal(out=den[:], in_=den[:])
                nc.vector.tensor_tensor(
                    out=e[:], in0=e[:], in1=den[:].to_broadcast([P, K]),
                    op=mybir.AluOpType.mult)
                prod = gp.tile([P, cfg.OUT * K], f32, tag="prod1")
                pv = prod[:].rearrange("p (c k) -> p c k", k=K)
                e_ap = e[:]
                al_b = bass.AP(e_ap.tensor, e_ap.offset,
                               [e_ap.ap[0], [0, cfg.OUT], e_ap.ap[1]])
                nc.vector.tensor_tensor(
                    out=pv,
                    in0=al_b,
                    in1=gv[:, :, 0:cfg.OUT].rearrange("p k c -> p c k"),
                    op=mybir.AluOpType.mult)
                of = gp.tile([P, cfg.OUT], f32, tag="of")
                nc.vector.tensor_reduce(out=of[:], in_=pv,
                                        axis=mybir.AxisListType.X,
                                        op=mybir.AluOpType.add)
                nc.vector.tensor_add(out=of[:], in0=of[:], in1=b1t[:])
                tmn = gp.tile([P, cfg.OUT], f32, tag="tmn1")
                nc.vector.tensor_scalar_min(out=tmn[:], in0=of[:], scalar1=0.0)
                nc.scalar.activation(out=tmn[:], in_=tmn[:],
                                     func=mybir.ActivationFunctionType.Exp)
                nc.vector.tensor_scalar_max(out=of[:], in0=of[:], scalar1=0.0)
                nc.vector.tensor_tensor(out=of[:], in0=of[:], in1=tmn[:],
                                        op=mybir.AluOpType.add)
                nc.vector.tensor_scalar_add(out=of[:], in0=of[:], scalar1=-1.0)
                nc.sync.dma_start(out=out_d[t * P:(t + 1) * P, :], in_=of[:])
    nc.finalize()
    return nc


def kernel(x, edge_index, W0, att_src0, att_dst0, b0, W1, att_src1, att_dst1, b1,
           _cfg=None):
    cfg = _cfg or Cfg()
    x = np.asarray(x, dtype=np.float32)
    W0 = np.asarray(W0, np.float32)
    W1 = np.asarray(W1, np.float32)
    att_src0 = np.asarray(att_src0, np.float32)
    att_dst0 = np.asarray(att_dst0, np.float32)
    att_src1 = np.asarray(att_src1, np.float32)
    att_dst1 = np.asarray(att_dst1, np.float32)
    b0 = np.asarray(b0, np.float32)
    b1 = np.asarray(b1, np.float32)

    assert cfg.n_pad > cfg.n_per, "need at least one pad row for dummy slots"
    perm_l, idx_cols, src_cols, kmax_l = _prep(cfg, x, edge_index)
    # unify per-tile K across devices (SPMD: one program)
    blocks = cfg.blocks
    kmax = [max(kmax_l[d][t] for d in range(NCORES)) for t in range(blocks)]
    ncols = int(np.sum(kmax))
    hp2 = cfg.n_pad // 2  # device-0 pad row in half-major layout
    dummy = (cfg.n_per // hp2) * NCORES * hp2 + (cfg.n_per % hp2)
    x_bf = x.astype(ml_dtypes.bfloat16)
    xz = np.zeros((1, cfg.IN), ml_dtypes.bfloat16)
    x_ext = np.concatenate([x_bf, xz])     # row N = zeros for pad slots
    idx_u, xg_u, msk_u = [], [], []
    for d in range(NCORES):
        buf = np.full((P, ncols), dummy, dtype=np.int32)
        sbuf_ = np.full((P, ncols), cfg.N, dtype=np.int64)   # pad -> zero row
        c_s = 0
        c_d = 0
        for t in range(blocks):
            kd = kmax_l[d][t]
            buf[:, c_d:c_d + kd] = idx_cols[d][:, c_s:c_s + kd]
            s = src_cols[d][:, c_s:c_s + kd]
            sbuf_[:, c_d:c_d + kd] = np.where(s < 0, cfg.N, s)
            c_s += kd
            c_d += kmax[t]
        idx_u.append(buf)
        msk_u.append(np.where(sbuf_ == cfg.N, PAD_AS, 0.0).astype(np.float32))
        # xg: per column block [128 feat, 128 dst] = x[src].T
        xgd = x_ext[sbuf_.T.reshape(-1)]           # [ncols*128, IN] bf16
        xgd = xgd.reshape(ncols, P, cfg.IN).transpose(0, 2, 1)  # [ncols, IN, P]
        xg_u.append(np.ascontiguousarray(
            xgd.transpose(1, 0, 2).reshape(cfg.IN, ncols * P)))

    # weights: A blockdiag for layer0 attention
    H, C = cfg.H, cfg.C
    A_src = np.zeros((H * C, H), np.float32)
    A_dst = np.zeros((H * C, H), np.float32)
    for h in range(H):
        A_src[h * C:(h + 1) * C, h] = att_src0[h]
        A_dst[h * C:(h + 1) * C, h] = att_dst0[h]
    w0cat = np.concatenate([W0, W0 @ A_src, W0 @ A_dst], axis=1)  # [IN, 80]
    w1cat = np.zeros((P, cfg.OUT + 2), np.float32)
    w1cat[:cfg.OUT] = np.concatenate(
        [W1, W1 @ att_src1[0][:, None], W1 @ att_dst1[0][:, None]], axis=1)
    biases = np.stack([b0.reshape(-1), b1.reshape(-1)])

    nc = _build(cfg, kmax, ncols)
    in_maps = []
    for d in range(NCORES):
        xt = np.zeros((cfg.IN, cfg.n_pad), ml_dtypes.bfloat16)
        xt[:, :cfg.n_per] = x_bf[perm_l[d]].T
        in_maps.append({
            "xt": xt,
            "xg": xg_u[d],
            "msk": msk_u[d],
            "idx": idx_u[d],
            "w0": w0cat.astype(ml_dtypes.bfloat16),
            "w1": w1cat,
            "bias": biases,
        })
    res = run_bass_kernel_spmd(nc, in_maps, core_ids=list(range(NCORES)))
    out = np.empty((cfg.N, cfg.OUT), np.float32)
    for d in range(NCORES):
        out[perm_l[d]] = res.results[d]["out"][:cfg.n_per]
    return out


# revision 15
# speedup vs baseline: 1.2884x; 1.0053x over previous
"""2-layer GAT on 8 NeuronCores (Trainium2, Bass/Tile).

Strategy (dst-sharded graph parallel):
  - Each core owns 12500 dst nodes (padded to 12544 = 98*128).
  - Layer 0: NO device-side gather. The host pre-gathers x rows per edge
    slot into transposed [128-feat, 128-dst] bf16 blocks; the device
    streams them sequentially (HWDGE, full bandwidth) and computes each
    slot's [feats|a_s] = x_src @ W0cat directly on the PE (one matmul per
    slot column, PSUM-batched 7 columns per bank). Pad slots are zero
    columns killed by an additive -1e30 logit mask.
  - Layer 1: node table rows [hW1|a_s1] (f32), AllGather (half overlap),
    per-slot-column indirect DMA gather (SWDGE), same softmax pipeline.
  - Segment softmax without max-subtraction (logits are O(10), exact
    same alpha ratios), weighted sum on DVE, ELU composed from
    min/exp/max, layer-1 projection fused per tile on the PE.
"""
import sys

sys.path.insert(0, "/opt/trn_rl_repo")

import numpy as np
import ml_dtypes

import concourse.bass as bass
import concourse.mybir as mybir
import concourse.tile as tile
from concourse import bacc
from concourse.masks import make_identity
from concourse.bass_utils import run_bass_kernel_spmd

P = 128
NCORES = 8
NEG_SLOPE = 0.2
PAD_AS = -1e30
MASK0 = -300.0   # L0 pad-slot logit mask: lrelu -> -60, exp -> 9e-27 (no 0/0)
MMB = 7          # slot-matmul columns batched per PSUM bank (7*72 <= 512 f32)


class Cfg:
    def __init__(self, n=100000, e=1600000, fin=128, heads=8, ch=8, out=64):
        self.N, self.E, self.IN, self.H, self.C, self.OUT = n, e, fin, heads, ch, out
        self.n_per = n // NCORES                      # owned real nodes
        self.blocks = (self.n_per + P - 1) // P       # tiles per device
        self.n_pad = self.blocks * P                  # padded nodes per device
        self.rows = NCORES * self.n_pad               # table rows
        self.w0cols = out + heads + heads             # feats | a_s | a_d
        self.t0cols = out + heads                     # slot cols layer0
        self.t1cols = out + 1                         # table row cols layer1


def _prep(cfg, x, edge_index):
    """Host-side sharding: permutation, per-device tiles, gather indices."""
    N, n_per, blocks, n_pad = cfg.N, cfg.n_per, cfg.blocks, cfg.n_pad
    src = np.asarray(edge_index[0], dtype=np.int64)
    dst = np.asarray(edge_index[1], dtype=np.int64)
    loops = np.arange(N, dtype=np.int64)
    src = np.concatenate([src, loops])
    dst = np.concatenate([dst, loops])

    owner = dst // n_per
    deg = np.bincount(dst, minlength=N)

    perm_l = []          # perm_l[d] = original node ids in canonical order
    idx_cols = []        # per-device int32 [P, C] gather indices (layer 1)
    src_cols = []        # per-device int64 [P, C] raw src node ids (-1 = pad)
    kmax_l = []          # per-device list of K per tile
    g_of = np.empty(N, dtype=np.int64)   # original node -> global table row
    for d in range(NCORES):
        lo, hi = d * n_per, (d + 1) * n_per
        nodes = np.arange(lo, hi)
        order = np.argsort(-deg[lo:hi], kind="stable")
        canon = nodes[order]                       # canonical order, len n_per
        perm_l.append(canon)
        cpos = np.arange(n_per)
        hp = n_pad // 2
        g_of[canon] = (cpos // hp) * NCORES * hp + d * hp + (cpos % hp)

    # per-device edge slots
    for d in range(NCORES):
        lo, hi = d * n_per, (d + 1) * n_per
        m = owner == d
        es, ed = src[m], dst[m]
        pos = np.empty(n_per, dtype=np.int64)
        pos[(perm_l[d] - lo)] = np.arange(n_per)
        ep = pos[ed - lo]                          # canonical pos of each edge's dst
        order = np.argsort(ep, kind="stable")
        es, ep = es[order], ep[order]
        counts = np.bincount(ep, minlength=n_pad)
        starts = np.concatenate([[0], np.cumsum(counts)])
        kmax = []
        cols = []
        scols = []
        hp2 = n_pad // 2                           # device-0 pad row (a_s=-1e30)
        dummy = (cfg.n_per // hp2) * NCORES * hp2 + 0 * hp2 + (cfg.n_per % hp2)
        for t in range(blocks):
            c = counts[t * P:(t + 1) * P]
            K = max(1, int(c.max()))
            kmax.append(K)
            tilecols = np.full((P, K), dummy, dtype=np.int32)
            tsrc = np.full((P, K), -1, dtype=np.int64)
            for p in range(P):
                node = t * P + p
                s0, s1 = starts[node], starts[node + 1]
                if s1 > s0:
                    tilecols[p, :s1 - s0] = g_of[es[s0:s1]]
                    tsrc[p, :s1 - s0] = es[s0:s1]
            cols.append(tilecols)
            scols.append(tsrc)
        idx_cols.append(np.concatenate(cols, axis=1))  # [P, sum K]
        src_cols.append(np.concatenate(scols, axis=1))
        kmax_l.append(kmax)
    return perm_l, idx_cols, src_cols, kmax_l


def _build(cfg, kmax, ncols):
    H, C, OUT = cfg.H, cfg.C, cfg.OUT
    n_pad, blocks, rows = cfg.n_pad, cfg.blocks, cfg.rows
    T0, T1 = cfg.t0cols, cfg.t1cols
    f32 = mybir.dt.float32
    bf16 = mybir.dt.bfloat16

    nc = bacc.Bacc(num_devices=NCORES)
    xt = nc.declare_dram_parameter("xt", [cfg.IN, n_pad], bf16, isOutput=False)
    xg = nc.declare_dram_parameter("xg", [P, ncols * P], bf16, isOutput=False)
    msk = nc.declare_dram_parameter("msk", [P, ncols], f32, isOutput=False)
    idx = nc.declare_dram_parameter("idx", [P, ncols], mybir.dt.int32, isOutput=False)
    w0 = nc.declare_dram_parameter("w0", [cfg.IN, cfg.w0cols], bf16, isOutput=False)
    w1 = nc.declare_dram_parameter("w1", [P, cfg.OUT + 2], f32, isOutput=False)
    bias = nc.declare_dram_parameter("bias", [2, cfg.OUT], f32, isOutput=False)
    out_d = nc.declare_dram_parameter("out", [n_pad, cfg.OUT], f32, isOutput=True)

    ltab1 = nc.dram_tensor("ltab1", [n_pad, T1], f32)
    tab1 = nc.dram_tensor("tab1", [rows, T1], f32, addr_space="Shared")

    with tile.TileContext(nc) as tc:
        with (
            tc.tile_pool(name="persist", bufs=1) as pp,
            tc.tile_pool(name="work", bufs=3) as wp,
            tc.tile_pool(name="gat", bufs=2) as gp,
            tc.tile_pool(name="g1p", bufs=3) as g1p,
            tc.tile_pool(name="ps", bufs=2, space="PSUM") as psp,
            tc.tile_pool(name="ps_slot", bufs=2, space="PSUM") as pss,
        ):
            # ---- constants ----
            w0t = pp.tile([cfg.IN, cfg.w0cols], bf16)
            nc.sync.dma_start(out=w0t[:], in_=w0[:])
            w1t = pp.tile([P, cfg.OUT + 2], f32)
            nc.sync.dma_start(out=w1t[:], in_=w1[:])
            b0t = pp.tile([P, cfg.OUT], f32)
            nc.sync.dma_start(out=b0t[:], in_=bias[0:1, :].to_broadcast([P, cfg.OUT]))
            b1t = pp.tile([P, cfg.OUT], f32)
            nc.sync.dma_start(out=b1t[:], in_=bias[1:2, :].to_broadcast([P, cfg.OUT]))
            idxt = pp.tile([P, ncols], mybir.dt.int32)
            nc.sync.dma_start(out=idxt[:], in_=idx[:])
            mskt = pp.tile([P, ncols], f32)
            nc.sync.dma_start(out=mskt[:], in_=msk[:])
            a_d0 = pp.tile([P, blocks * H], f32)
            a_d1 = pp.tile([P, blocks], f32)
            ident = pp.tile([P, P], f32)
            make_identity(nc, ident[:])
            pad_as = pp.tile([P, H], f32)
            nc.vector.memset(pad_as[:], PAD_AS)

            hp = n_pad // 2
            half_t = hp // P

            # ---- P1: a_d0 per node (tiny matmuls) ----
            for t in range(blocks):
                xs = wp.tile([cfg.IN, P], bf16, tag="xs")
                nc.sync.dma_start(out=xs[:], in_=xt[:, t * P:(t + 1) * P])
                ps = psp.tile([P, H], f32, tag="mm0")
                nc.tensor.matmul(out=ps[:], lhsT=xs[:],
                                 rhs=w0t[:, T0:T0 + H],
                                 start=True, stop=True)
                nc.vector.tensor_copy(out=a_d0[:, t * H:(t + 1) * H], in_=ps[:])

            # ---- L0 edge phase (host-pregathered slots) + fused L1 proj ----
            col = 0
            for t in range(blocks):
                K = kmax[t]
                g = gp.tile([P, K * T0], f32, tag="g0")
                for k0 in range(0, K, MMB):
                    cnt = min(MMB, K - k0)
                    xgc = wp.tile([P, cnt * P], bf16, tag="xgc")
                    nc.sync.dma_start(
                        out=xgc[:],
                        in_=xg[:, (col + k0) * P:(col + k0 + cnt) * P])
                    ps = pss.tile([P, cnt * T0], f32, tag="slotmm")
                    for j in range(cnt):
                        nc.tensor.matmul(
                            out=ps[:, j * T0:(j + 1) * T0],
                            lhsT=xgc[:, j * P:(j + 1) * P],
                            rhs=w0t[:, 0:T0], start=True, stop=True)
                    nc.scalar.copy(out=g[:, k0 * T0:(k0 + cnt) * T0], in_=ps[:])
                gv = g[:].rearrange("p (k w) -> p k w", w=T0)
                # e[p,h,k] = a_s[src] + a_d[dst] + mask
                e = gp.tile([P, H * K], f32, tag="e")
                ev = e[:].rearrange("p (h k) -> p h k", k=K)
                asg = gv[:, :, cfg.OUT:T0].rearrange("p k h -> p h k")
                nc.vector.tensor_tensor(
                    out=ev, in0=asg,
                    in1=a_d0[:, t * H:(t + 1) * H].to_broadcast([P, H, K]),
                    op=mybir.AluOpType.add)
                mv = mskt[:, col:col + K]
                mb = bass.AP(mv.tensor, mv.offset,
                             [mv.ap[0], [0, H], mv.ap[1]])
                nc.vector.tensor_tensor(out=ev, in0=ev, in1=mb,
                                        op=mybir.AluOpType.add)
                col += K
                scr = gp.tile([P, H * K], f32, tag="scr")
                nc.vector.tensor_scalar_mul(out=scr[:], in0=e[:],
                                             scalar1=NEG_SLOPE)
                nc.vector.tensor_tensor(out=e[:], in0=e[:], in1=scr[:],
                                        op=mybir.AluOpType.max)
                nc.scalar.activation(out=e[:], in_=e[:],
                                     func=mybir.ActivationFunctionType.Exp)
                den = gp.tile([P, H], f32, tag="den")
                nc.vector.tensor_reduce(out=den[:], in_=ev,
                                        axis=mybir.AxisListType.X,
                                        op=mybir.AluOpType.add)
                nc.vector.reciprocal(out=den[:], in_=den[:])
                prod = gp.tile([P, cfg.OUT * K], f32, tag="prod")
                pv = prod[:].rearrange("p (h c k) -> p h c k", c=C, k=K)
                al_b = bass.AP(ev.tensor, ev.offset,
                               [ev.ap[0], ev.ap[1], [0, C], ev.ap[2]])
                nc.vector.tensor_tensor(
                    out=pv,
                    in0=al_b,
                    in1=gv[:, :, 0:cfg.OUT].rearrange(
                        "p k (h c) -> p h c k", c=C),
                    op=mybir.AluOpType.mult)
                hfeat = gp.tile([P, cfg.OUT], f32, tag="hfeat")
                nc.vector.tensor_reduce(
                    out=hfeat[:], in_=pv, axis=mybir.AxisListType.X,
                    op=mybir.AluOpType.add)
                # normalize by softmax denom after the K-reduction (64 < H*K)
                dnb = bass.AP(den[:].tensor, den[:].offset,
                              [den[:].ap[0], den[:].ap[1], [0, C]])
                hv = hfeat[:].rearrange("p (h c) -> p h c", c=C)
                nc.vector.tensor_tensor(out=hv, in0=hv, in1=dnb,
                                        op=mybir.AluOpType.mult)
                nc.vector.tensor_add(out=hfeat[:], in0=hfeat[:], in1=b0t[:])
                # ELU: h = (max(x,0) - 1) + exp(min(x,0))
                tmn = gp.tile([P, cfg.OUT], f32, tag="tmn")
                nc.vector.tensor_scalar_min(out=tmn[:], in0=hfeat[:], scalar1=0.0)
                nc.scalar.activation(out=tmn[:], in_=tmn[:],
                                     func=mybir.ActivationFunctionType.Exp)
                nc.vector.tensor_scalar(
                    out=hfeat[:], in0=hfeat[:], scalar1=0.0, scalar2=-1.0,
                    op0=mybir.AluOpType.max, op1=mybir.AluOpType.add)
                nc.vector.tensor_tensor(out=hfeat[:], in0=hfeat[:], in1=tmn[:],
                                        op=mybir.AluOpType.add)
                # L1 projection: rows of ltab1 = [h @ W1 | h @ w_src1]; a_d1 kept
                pst = psp.tile([P, P], f32, tag="tr")
                nc.tensor.transpose(out=pst[:cfg.OUT, :], in_=hfeat[:],
                                    identity=ident[:])
                ht = wp.tile([cfg.OUT, P], f32, tag="ht")
                nc.scalar.copy(out=ht[:], in_=pst[:cfg.OUT, :])
                ps1 = psp.tile([P, cfg.OUT + 2], f32, tag="mm1")
                nc.tensor.matmul(out=ps1[:], lhsT=ht[:],
                                 rhs=w1t[:cfg.OUT, :], start=True, stop=True)
                row1 = wp.tile([P, T1], f32, tag="row1")
                nc.scalar.copy(out=row1[:], in_=ps1[:, 0:T1])
                nc.vector.tensor_copy(out=a_d1[:, t:t + 1],
                                      in_=ps1[:, T1:T1 + 1])
                nc.sync.dma_start(out=ltab1[t * P:(t + 1) * P, :], in_=row1[:])
                if t == half_t - 1:
                    nc.gpsimd.collective_compute(
                        "AllGather", mybir.AluOpType.bypass,
                        replica_groups=[list(range(NCORES))],
                        ins=[ltab1[0:hp, :]],
                        outs=[tab1[0:NCORES * hp, :]])
            npad_rows = n_pad - cfg.n_per
            if npad_rows > 0:
                nc.sync.dma_start(
                    out=ltab1[cfg.n_per:n_pad, cfg.OUT:cfg.OUT + 1],
                    in_=pad_as[0:npad_rows, 0:1])

            # ---- AllGather layer-1 table (2nd half) ----
            nc.gpsimd.collective_compute(
                "AllGather", mybir.AluOpType.bypass,
                replica_groups=[list(range(NCORES))],
                ins=[ltab1[hp:n_pad, :]],
                outs=[tab1[NCORES * hp:rows, :]])

            # ---- L1 edge phase ----
            col = 0
            for t in range(blocks):
                K = kmax[t]
                g = g1p.tile([P, K * T1], f32, tag="g1")
                for k in range(K):
                    nc.gpsimd.indirect_dma_start(
                        out=g[:, k * T1:(k + 1) * T1],
                        out_offset=None,
                        in_=tab1[:, :],
                        in_offset=bass.IndirectOffsetOnAxis(
                            ap=idxt[:, col + k:col + k + 1], axis=0))
                col += K
                gv = g[:].rearrange("p (k w) -> p k w", w=T1)
                e = gp.tile([P, K], f32, tag="e1")
                asg1 = gv[:, :, cfg.OUT:T1].rearrange("p k w -> p (k w)")
                nc.vector.tensor_tensor(
                    out=e[:], in0=asg1,
                    in1=a_d1[:, t:t + 1].to_broadcast([P, K]),
                    op=mybir.AluOpType.add)
                scr1 = gp.tile([P, K], f32, tag="scr1")
                nc.vector.tensor_scalar_mul(out=scr1[:], in0=e[:],
                                             scalar1=NEG_SLOPE)
                nc.vector.tensor_tensor(out=e[:], in0=e[:], in1=scr1[:],
                                        op=mybir.AluOpType.max)
                nc.scalar.activation(out=e[:], in_=e[:],
                                     func=mybir.ActivationFunctionType.Exp)
                den = gp.tile([P, 1], f32, tag="den1")
                nc.vector.tensor_reduce(out=den[:], in_=e[:],
                                        axis=mybir.AxisListType.X,
                                        op=mybir.AluOpType.add)
                nc.vector.reciprocal(out=den[:], in_=den[:])
                nc.vector.tensor_tensor(
                    out=e[:], in0=e[:], in1=den[:].to_broadcast([P, K]),
                    op=mybir.AluOpType.mult)
                prod = gp.tile([P, cfg.OUT * K], f32, tag="prod1")
                pv = prod[:].rearrange("p (c k) -> p c k", k=K)
                e_ap = e[:]
                al_b = bass.AP(e_ap.tensor, e_ap.offset,
                               [e_ap.ap[0], [0, cfg.OUT], e_ap.ap[1]])
                nc.vector.tensor_tensor(
                    out=pv,
                    in0=al_b,
                    in1=gv[:, :, 0:cfg.OUT].rearrange("p k c -> p c k"),
                    op=mybir.AluOpType.mult)
                of = gp.tile([P, cfg.OUT], f32, tag="of")
                nc.vector.tensor_reduce(out=of[:], in_=pv,
                                        axis=mybir.AxisListType.X,
                                        op=mybir.AluOpType.add)
                nc.vector.tensor_add(out=of[:], in0=of[:], in1=b1t[:])
                tmn = gp.tile([P, cfg.OUT], f32, tag="tmn1")
                nc.vector.tensor_scalar_min(out=tmn[:], in0=of[:], scalar1=0.0)
                nc.scalar.activation(out=tmn[:], in_=tmn[:],
                                     func=mybir.ActivationFunctionType.Exp)
                nc.vector.tensor_scalar(
                    out=of[:], in0=of[:], scalar1=0.0, scalar2=-1.0,
                    op0=mybir.AluOpType.max, op1=mybir.AluOpType.add)
                nc.vector.tensor_tensor(out=of[:], in0=of[:], in1=tmn[:],
                                        op=mybir.AluOpType.add)
                nc.sync.dma_start(out=out_d[t * P:(t + 1) * P, :], in_=of[:])
    nc.finalize()
    return nc


def kernel(x, edge_index, W0, att_src0, att_dst0, b0, W1, att_src1, att_dst1, b1,
           _cfg=None):
    cfg = _cfg or Cfg()
    x = np.asarray(x, dtype=np.float32)
    W0 = np.asarray(W0, np.float32)
    W1 = np.asarray(W1, np.float32)
    att_src0 = np.asarray(att_src0, np.float32)
    att_dst0 = np.asarray(att_dst0, np.float32)
    att_src1 = np.asarray(att_src1, np.float32)
    att_dst1 = np.asarray(att_dst1, np.float32)
    b0 = np.asarray(b0, np.float32)
    b1 = np.asarray(b1, np.float32)

    assert cfg.n_pad > cfg.n_per, "need at least one pad row for dummy slots"
    perm_l, idx_cols, src_cols, kmax_l = _prep(cfg, x, edge_index)
    # unify per-tile K across devices (SPMD: one program)
    blocks = cfg.blocks
    kmax = [max(kmax_l[d][t] for d in range(NCORES)) for t in range(blocks)]
    ncols = int(np.sum(kmax))
    hp2 = cfg.n_pad // 2  # device-0 pad row in half-major layout
    dummy = (cfg.n_per // hp2) * NCORES * hp2 + (cfg.n_per % hp2)
    x_bf = x.astype(ml_dtypes.bfloat16)
    xz = np.zeros((1, cfg.IN), ml_dtypes.bfloat16)
    x_ext = np.concatenate([x_bf, xz])     # row N = zeros for pad slots
    idx_u, xg_u, msk_u = [], [], []
    for d in range(NCORES):
        buf = np.full((P, ncols), dummy, dtype=np.int32)
        sbuf_ = np.full((P, ncols), cfg.N, dtype=np.int64)   # pad -> zero row
        c_s = 0
        c_d = 0
        for t in range(blocks):
            kd = kmax_l[d][t]
            buf[:, c_d:c_d + kd] = idx_cols[d][:, c_s:c_s + kd]
            s = src_cols[d][:, c_s:c_s + kd]
            sbuf_[:, c_d:c_d + kd] = np.where(s < 0, cfg.N, s)
            c_s += kd
            c_d += kmax[t]
        idx_u.append(buf)
        msk_u.append(np.where(sbuf_ == cfg.N, MASK0, 0.0).astype(np.float32))
        # xg: per column block [128 feat, 128 dst] = x[src].T
        xgd = x_ext[sbuf_.T.reshape(-1)]           # [ncols*128, IN] bf16
        xgd = xgd.reshape(ncols, P, cfg.IN).transpose(0, 2, 1)  # [ncols, IN, P]
        xg_u.append(np.ascontiguousarray(
            xgd.transpose(1, 0, 2).reshape(cfg.IN, ncols * P)))

    # weights: A blockdiag for layer0 attention
    H, C = cfg.H, cfg.C
    A_src = np.zeros((H * C, H), np.float32)
    A_dst = np.zeros((H * C, H), np.float32)
    for h in range(H):
        A_src[h * C:(h + 1) * C, h] = att_src0[h]
        A_dst[h * C:(h + 1) * C, h] = att_dst0[h]
    w0cat = np.concatenate([W0, W0 @ A_src, W0 @ A_dst], axis=1)  # [IN, 80]
    w1cat = np.zeros((P, cfg.OUT + 2), np.float32)
    w1cat[:cfg.OUT] = np.concatenate(
        [W1, W1 @ att_src1[0][:, None], W1 @ att_dst1[0][:, None]], axis=1)
    biases = np.stack([b0.reshape(-1), b1.reshape(-1)])

    nc = _build(cfg, kmax, ncols)
    in_maps = []
    for d in range(NCORES):
        xt = np.zeros((cfg.IN, cfg.n_pad), ml_dtypes.bfloat16)
        xt[:, :cfg.n_per] = x_bf[perm_l[d]].T
        in_maps.append({
            "xt": xt,
            "xg": xg_u[d],
            "msk": msk_u[d],
            "idx": idx_u[d],
            "w0": w0cat.astype(ml_dtypes.bfloat16),
            "w1": w1cat,
            "bias": biases,
        })
    res = run_bass_kernel_spmd(nc, in_maps, core_ids=list(range(NCORES)))
    out = np.empty((cfg.N, cfg.OUT), np.float32)
    for d in range(NCORES):
        out[perm_l[d]] = res.results[d]["out"][:cfg.n_per]
    return out


# revision 17
# speedup vs baseline: 1.2887x; 1.0002x over previous
"""2-layer GAT on 8 NeuronCores (Trainium2, Bass/Tile).

Strategy (dst-sharded graph parallel):
  - Each core owns 12500 dst nodes (padded to 12544 = 98*128).
  - Layer 0: NO device-side gather. The host pre-gathers x rows per edge
    slot into transposed [128-feat, 128-dst] bf16 blocks; the device
    streams them sequentially (HWDGE, full bandwidth) and computes each
    slot's [feats|a_s] = x_src @ W0cat directly on the PE (one matmul per
    slot column, PSUM-batched 7 columns per bank). Pad slots are zero
    columns killed by an additive -1e30 logit mask.
  - Layer 1: node table rows [hW1|a_s1] (f32), AllGather (half overlap),
    per-slot-column indirect DMA gather (SWDGE), same softmax pipeline.
  - Segment softmax without max-subtraction (logits are O(10), exact
    same alpha ratios), weighted sum on DVE, ELU composed from
    min/exp/max, layer-1 projection fused per tile on the PE.
"""
import sys

sys.path.insert(0, "/opt/trn_rl_repo")

import numpy as np
import ml_dtypes

import concourse.bass as bass
import concourse.mybir as mybir
import concourse.tile as tile
from concourse import bacc
from concourse.masks import make_identity
from concourse.bass_utils import run_bass_kernel_spmd

P = 128
NCORES = 8
NEG_SLOPE = 0.2
PAD_AS = -1e30
MASK0 = -300.0   # L0 pad-slot logit mask: lrelu -> -60, exp -> 9e-27 (no 0/0)
MMB = 7          # slot-matmul columns batched per PSUM bank (7*72 <= 512 f32)


class Cfg:
    def __init__(self, n=100000, e=1600000, fin=128, heads=8, ch=8, out=64):
        self.N, self.E, self.IN, self.H, self.C, self.OUT = n, e, fin, heads, ch, out
        self.n_per = n // NCORES                      # owned real nodes
        self.blocks = (self.n_per + P - 1) // P       # tiles per device
        self.n_pad = self.blocks * P                  # padded nodes per device
        self.rows = NCORES * self.n_pad               # table rows
        self.w0cols = out + heads + heads             # feats | a_s | a_d
        self.t0cols = out + heads                     # slot cols layer0
        self.t1cols = out + 1                         # table row cols layer1


def _prep(cfg, x, edge_index):
    """Host-side sharding: permutation, per-device tiles, gather indices."""
    N, n_per, blocks, n_pad = cfg.N, cfg.n_per, cfg.blocks, cfg.n_pad
    src = np.asarray(edge_index[0], dtype=np.int64)
    dst = np.asarray(edge_index[1], dtype=np.int64)
    loops = np.arange(N, dtype=np.int64)
    src = np.concatenate([src, loops])
    dst = np.concatenate([dst, loops])

    owner = dst // n_per
    deg = np.bincount(dst, minlength=N)

    perm_l = []          # perm_l[d] = original node ids in canonical order
    idx_cols = []        # per-device int32 [P, C] gather indices (layer 1)
    src_cols = []        # per-device int64 [P, C] raw src node ids (-1 = pad)
    kmax_l = []          # per-device list of K per tile
    g_of = np.empty(N, dtype=np.int64)   # original node -> global table row
    for d in range(NCORES):
        lo, hi = d * n_per, (d + 1) * n_per
        nodes = np.arange(lo, hi)
        order = np.argsort(-deg[lo:hi], kind="stable")
        canon = nodes[order]                       # canonical order, len n_per
        perm_l.append(canon)
        cpos = np.arange(n_per)
        hp = n_pad // 2
        g_of[canon] = (cpos // hp) * NCORES * hp + d * hp + (cpos % hp)

    # per-device edge slots
    for d in range(NCORES):
        lo, hi = d * n_per, (d + 1) * n_per
        m = owner == d
        es, ed = src[m], dst[m]
        pos = np.empty(n_per, dtype=np.int64)
        pos[(perm_l[d] - lo)] = np.arange(n_per)
        ep = pos[ed - lo]                          # canonical pos of each edge's dst
        order = np.argsort(ep, kind="stable")
        es, ep = es[order], ep[order]
        counts = np.bincount(ep, minlength=n_pad)
        starts = np.concatenate([[0], np.cumsum(counts)])
        kmax = []
        cols = []
        scols = []
        hp2 = n_pad // 2                           # device-0 pad row (a_s=-1e30)
        dummy = (cfg.n_per // hp2) * NCORES * hp2 + 0 * hp2 + (cfg.n_per % hp2)
        for t in range(blocks):
            c = counts[t * P:(t + 1) * P]
            K = max(1, int(c.max()))
            kmax.append(K)
            tilecols = np.full((P, K), dummy, dtype=np.int32)
            tsrc = np.full((P, K), -1, dtype=np.int64)
            for p in range(P):
                node = t * P + p
                s0, s1 = starts[node], starts[node + 1]
                if s1 > s0:
                    tilecols[p, :s1 - s0] = g_of[es[s0:s1]]
                    tsrc[p, :s1 - s0] = es[s0:s1]
            cols.append(tilecols)
            scols.append(tsrc)
        idx_cols.append(np.concatenate(cols, axis=1))  # [P, sum K]
        src_cols.append(np.concatenate(scols, axis=1))
        kmax_l.append(kmax)
    return perm_l, idx_cols, src_cols, kmax_l


def _build(cfg, kmax, ncols):
    H, C, OUT = cfg.H, cfg.C, cfg.OUT
    n_pad, blocks, rows = cfg.n_pad, cfg.blocks, cfg.rows
    T0, T1 = cfg.t0cols, cfg.t1cols
    f32 = mybir.dt.float32
    bf16 = mybir.dt.bfloat16

    nc = bacc.Bacc(num_devices=NCORES)
    xt = nc.declare_dram_parameter("xt", [cfg.IN, n_pad], bf16, isOutput=False)
    xg = nc.declare_dram_parameter("xg", [P, ncols * P], bf16, isOutput=False)
    msk = nc.declare_dram_parameter("msk", [P, ncols], f32, isOutput=False)
    idx = nc.declare_dram_parameter("idx", [P, ncols], mybir.dt.int32, isOutput=False)
    w0 = nc.declare_dram_parameter("w0", [cfg.IN, cfg.w0cols], bf16, isOutput=False)
    w1 = nc.declare_dram_parameter("w1", [P, cfg.OUT + 2], f32, isOutput=False)
    bias = nc.declare_dram_parameter("bias", [2, cfg.OUT], f32, isOutput=False)
    out_d = nc.declare_dram_parameter("out", [n_pad, cfg.OUT], f32, isOutput=True)

    ltab1 = nc.dram_tensor("ltab1", [n_pad, T1], f32)
    tab1 = nc.dram_tensor("tab1", [rows, T1], f32, addr_space="Shared")

    with tile.TileContext(nc) as tc:
        with (
            tc.tile_pool(name="persist", bufs=1) as pp,
            tc.tile_pool(name="work", bufs=3) as wp,
            tc.tile_pool(name="gat", bufs=2) as gp,
            tc.tile_pool(name="g1p", bufs=3) as g1p,
            tc.tile_pool(name="ps", bufs=2, space="PSUM") as psp,
            tc.tile_pool(name="ps_slot", bufs=2, space="PSUM") as pss,
        ):
            # ---- constants ----
            w0t = pp.tile([cfg.IN, cfg.w0cols], bf16)
            nc.sync.dma_start(out=w0t[:], in_=w0[:])
            w1t = pp.tile([P, cfg.OUT + 2], f32)
            nc.sync.dma_start(out=w1t[:], in_=w1[:])
            b0t = pp.tile([P, cfg.OUT], f32)
            nc.sync.dma_start(out=b0t[:], in_=bias[0:1, :].to_broadcast([P, cfg.OUT]))
            b1t = pp.tile([P, cfg.OUT], f32)
            nc.sync.dma_start(out=b1t[:], in_=bias[1:2, :].to_broadcast([P, cfg.OUT]))
            idxt = pp.tile([P, ncols], mybir.dt.int32)
            nc.sync.dma_start(out=idxt[:], in_=idx[:])
            mskt = pp.tile([P, ncols], f32)
            nc.sync.dma_start(out=mskt[:], in_=msk[:])
            a_d0 = pp.tile([P, blocks * H], f32)
            a_d1 = pp.tile([P, blocks], f32)
            ident = pp.tile([P, P], f32)
            make_identity(nc, ident[:])
            pad_as = pp.tile([P, H], f32)
            nc.vector.memset(pad_as[:], PAD_AS)

            hp = n_pad // 2
            half_t = hp // P

            # ---- P1: a_d0 per node (tiny matmuls) ----
            for t in range(blocks):
                xs = wp.tile([cfg.IN, P], bf16, tag="xs")
                nc.sync.dma_start(out=xs[:], in_=xt[:, t * P:(t + 1) * P])
                ps = psp.tile([P, H], f32, tag="mm0")
                nc.tensor.matmul(out=ps[:], lhsT=xs[:],
                                 rhs=w0t[:, T0:T0 + H],
                                 start=True, stop=True)
                nc.vector.tensor_copy(out=a_d0[:, t * H:(t + 1) * H], in_=ps[:])

            # ---- L0 edge phase (host-pregathered slots) + fused L1 proj ----
            col = 0
            for t in range(blocks):
                K = kmax[t]
                g = gp.tile([P, K * T0], f32, tag="g0")
                for k0 in range(0, K, MMB):
                    cnt = min(MMB, K - k0)
                    xgc = wp.tile([P, cnt * P], bf16, tag="xgc")
                    nc.sync.dma_start(
                        out=xgc[:],
                        in_=xg[:, (col + k0) * P:(col + k0 + cnt) * P])
                    ps = pss.tile([P, cnt * T0], f32, tag="slotmm")
                    for j in range(cnt):
                        nc.tensor.matmul(
                            out=ps[:, j * T0:(j + 1) * T0],
                            lhsT=xgc[:, j * P:(j + 1) * P],
                            rhs=w0t[:, 0:T0], start=True, stop=True)
                    nc.scalar.copy(out=g[:, k0 * T0:(k0 + cnt) * T0], in_=ps[:])
                gv = g[:].rearrange("p (k w) -> p k w", w=T0)
                # e[p,h,k] = a_s[src] + a_d[dst] + mask
                e = gp.tile([P, H * K], f32, tag="e")
                ev = e[:].rearrange("p (h k) -> p h k", k=K)
                asg = gv[:, :, cfg.OUT:T0].rearrange("p k h -> p h k")
                nc.vector.tensor_tensor(
                    out=ev, in0=asg,
                    in1=a_d0[:, t * H:(t + 1) * H].to_broadcast([P, H, K]),
                    op=mybir.AluOpType.add)
                mv = mskt[:, col:col + K]
                mb = bass.AP(mv.tensor, mv.offset,
                             [mv.ap[0], [0, H], mv.ap[1]])
                nc.vector.tensor_tensor(out=ev, in0=ev, in1=mb,
                                        op=mybir.AluOpType.add)
                col += K
                scr = gp.tile([P, H * K], f32, tag="scr")
                nc.vector.tensor_scalar_mul(out=scr[:], in0=e[:],
                                             scalar1=NEG_SLOPE)
                nc.vector.tensor_tensor(out=e[:], in0=e[:], in1=scr[:],
                                        op=mybir.AluOpType.max)
                nc.scalar.activation(out=e[:], in_=e[:],
                                     func=mybir.ActivationFunctionType.Exp)
                den = gp.tile([P, H], f32, tag="den")
                nc.vector.tensor_reduce(out=den[:], in_=ev,
                                        axis=mybir.AxisListType.X,
                                        op=mybir.AluOpType.add)
                nc.vector.reciprocal(out=den[:], in_=den[:])
                prod = gp.tile([P, cfg.OUT * K], f32, tag="prod")
                pv = prod[:].rearrange("p (h c k) -> p h c k", c=C, k=K)
                al_b = bass.AP(ev.tensor, ev.offset,
                               [ev.ap[0], ev.ap[1], [0, C], ev.ap[2]])
                nc.vector.tensor_tensor(
                    out=pv,
                    in0=al_b,
                    in1=gv[:, :, 0:cfg.OUT].rearrange(
                        "p k (h c) -> p h c k", c=C),
                    op=mybir.AluOpType.mult)
                hfeat = gp.tile([P, cfg.OUT], f32, tag="hfeat")
                nc.vector.tensor_reduce(
                    out=hfeat[:], in_=pv, axis=mybir.AxisListType.X,
                    op=mybir.AluOpType.add)
                # normalize by softmax denom after the K-reduction (64 < H*K)
                dnb = bass.AP(den[:].tensor, den[:].offset,
                              [den[:].ap[0], den[:].ap[1], [0, C]])
                hv = hfeat[:].rearrange("p (h c) -> p h c", c=C)
                nc.vector.tensor_tensor(out=hv, in0=hv, in1=dnb,
                                        op=mybir.AluOpType.mult)
                nc.vector.tensor_add(out=hfeat[:], in0=hfeat[:], in1=b0t[:])
                # ELU: h = (max(x,0) - 1) + exp(min(x,0))
                tmn = gp.tile([P, cfg.OUT], f32, tag="tmn")
                nc.vector.tensor_scalar_min(out=tmn[:], in0=hfeat[:], scalar1=0.0)
                nc.scalar.activation(out=tmn[:], in_=tmn[:],
                                     func=mybir.ActivationFunctionType.Exp)
                nc.vector.tensor_scalar(
                    out=hfeat[:], in0=hfeat[:], scalar1=0.0, scalar2=-1.0,
                    op0=mybir.AluOpType.max, op1=mybir.AluOpType.add)
                nc.vector.tensor_tensor(out=hfeat[:], in0=hfeat[:], in1=tmn[:],
                                        op=mybir.AluOpType.add)
                # L1 projection: rows of ltab1 = [h @ W1 | h @ w_src1]; a_d1 kept
                pst = psp.tile([P, P], f32, tag="tr")
                nc.tensor.transpose(out=pst[:cfg.OUT, :], in_=hfeat[:],
                                    identity=ident[:])
                ht = wp.tile([cfg.OUT, P], f32, tag="ht")
                nc.scalar.copy(out=ht[:], in_=pst[:cfg.OUT, :])
                ps1 = psp.tile([P, cfg.OUT + 2], f32, tag="mm1")
                nc.tensor.matmul(out=ps1[:], lhsT=ht[:],
                                 rhs=w1t[:cfg.OUT, :], start=True, stop=True)
                row1 = wp.tile([P, T1], f32, tag="row1")
                nc.scalar.copy(out=row1[:], in_=ps1[:, 0:T1])
                nc.vector.tensor_copy(out=a_d1[:, t:t + 1],
                                      in_=ps1[:, T1:T1 + 1])
                nc.sync.dma_start(out=ltab1[t * P:(t + 1) * P, :], in_=row1[:])
                if t == half_t - 1:
                    nc.gpsimd.collective_compute(
                        "AllGather", mybir.AluOpType.bypass,
                        replica_groups=[list(range(NCORES))],
                        ins=[ltab1[0:hp, :]],
                        outs=[tab1[0:NCORES * hp, :]])
            npad_rows = n_pad - cfg.n_per
            if npad_rows > 0:
                nc.sync.dma_start(
                    out=ltab1[cfg.n_per:n_pad, cfg.OUT:cfg.OUT + 1],
                    in_=pad_as[0:npad_rows, 0:1])

            # ---- AllGather layer-1 table (2nd half) ----
            nc.gpsimd.collective_compute(
                "AllGather", mybir.AluOpType.bypass,
                replica_groups=[list(range(NCORES))],
                ins=[ltab1[hp:n_pad, :]],
                outs=[tab1[NCORES * hp:rows, :]])

            # ---- L1 edge phase ----
            col = 0
            for t in range(blocks):
                K = kmax[t]
                g = g1p.tile([P, K * T1], f32, tag="g1")
                for k in range(K):
                    nc.gpsimd.indirect_dma_start(
                        out=g[:, k * T1:(k + 1) * T1],
                        out_offset=None,
                        in_=tab1[:, :],
                        in_offset=bass.IndirectOffsetOnAxis(
                            ap=idxt[:, col + k:col + k + 1], axis=0))
                col += K
                gv = g[:].rearrange("p (k w) -> p k w", w=T1)
                e = gp.tile([P, K], f32, tag="e1")
                asg1 = gv[:, :, cfg.OUT:T1].rearrange("p k w -> p (k w)")
                nc.vector.tensor_tensor(
                    out=e[:], in0=asg1,
                    in1=a_d1[:, t:t + 1].to_broadcast([P, K]),
                    op=mybir.AluOpType.add)
                scr1 = gp.tile([P, K], f32, tag="scr1")
                nc.vector.tensor_scalar_mul(out=scr1[:], in0=e[:],
                                             scalar1=NEG_SLOPE)
                nc.vector.tensor_tensor(out=e[:], in0=e[:], in1=scr1[:],
                                        op=mybir.AluOpType.max)
                nc.scalar.activation(out=e[:], in_=e[:],
                                     func=mybir.ActivationFunctionType.Exp)
                den = gp.tile([P, 1], f32, tag="den1")
                nc.vector.tensor_reduce(out=den[:], in_=e[:],
                                        axis=mybir.AxisListType.X,
                                        op=mybir.AluOpType.add)
                nc.vector.reciprocal(out=den[:], in_=den[:])
                nc.vector.tensor_tensor(
                    out=e[:], in0=e[:], in1=den[:].to_broadcast([P, K]),
                    op=mybir.AluOpType.mult)
                prod = gp.tile([P, cfg.OUT * K], f32, tag="prod1")
                pv = prod[:].rearrange("p (c k) -> p c k", k=K)
                e_ap = e[:]
                al_b = bass.AP(e_ap.tensor, e_ap.offset,
                               [e_ap.ap[0], [0, cfg.OUT], e_ap.ap[1]])
                nc.vector.tensor_tensor(
                    out=pv,
                    in0=al_b,
                    in1=gv[:, :, 0:cfg.OUT].rearrange("p k c -> p c k"),
                    op=mybir.AluOpType.mult)
                of = gp.tile([P, cfg.OUT], f32, tag="of")
                nc.vector.tensor_reduce(out=of[:], in_=pv,
                                        axis=mybir.AxisListType.X,
                                        op=mybir.AluOpType.add)
                nc.vector.tensor_add(out=of[:], in0=of[:], in1=b1t[:])
                tmn = gp.tile([P, cfg.OUT], f32, tag="tmn1")
                nc.vector.tensor_scalar_min(out=tmn[:], in0=of[:], scalar1=0.0)
                nc.scalar.activation(out=tmn[:], in_=tmn[:],
                                     func=mybir.ActivationFunctionType.Exp)
                nc.vector.tensor_scalar(
                    out=of[:], in0=of[:], scalar1=0.0, scalar2=-1.0,
                    op0=mybir.AluOpType.max, op1=mybir.AluOpType.add)
                nc.vector.tensor_tensor(out=of[:], in0=of[:], in1=tmn[:],
                                        op=mybir.AluOpType.add)
                nc.sync.dma_start(out=out_d[t * P:(t + 1) * P, :], in_=of[:])
    nc.finalize()
    return nc


def kernel(x, edge_index, W0, att_src0, att_dst0, b0, W1, att_src1, att_dst1, b1,
           _cfg=None):
    cfg = _cfg or Cfg()
    x = np.asarray(x, dtype=np.float32)
    W0 = np.asarray(W0, np.float32)
    W1 = np.asarray(W1, np.float32)
    att_src0 = np.asarray(att_src0, np.float32)
    att_dst0 = np.asarray(att_dst0, np.float32)
    att_src1 = np.asarray(att_src1, np.float32)
    att_dst1 = np.asarray(att_dst1, np.float32)
    b0 = np.asarray(b0, np.float32)
    b1 = np.asarray(b1, np.float32)

    assert cfg.n_pad > cfg.n_per, "need at least one pad row for dummy slots"
    perm_l, idx_cols, src_cols, kmax_l = _prep(cfg, x, edge_index)
    # unify per-tile K across devices (SPMD: one program)
    blocks = cfg.blocks
    kmax = [max(kmax_l[d][t] for d in range(NCORES)) for t in range(blocks)]
    ncols = int(np.sum(kmax))
    hp2 = cfg.n_pad // 2  # device-0 pad row in half-major layout
    dummy = (cfg.n_per // hp2) * NCORES * hp2 + (cfg.n_per % hp2)
    x_bf = x.astype(ml_dtypes.bfloat16)
    xz = np.zeros((1, cfg.IN), ml_dtypes.bfloat16)
    x_ext = np.concatenate([x_bf, xz])     # row N = zeros for pad slots
    idx_u, xg_u, msk_u = [], [], []
    for d in range(NCORES):
        buf = np.full((P, ncols), dummy, dtype=np.int32)
        sbuf_ = np.full((P, ncols), cfg.N, dtype=np.int64)   # pad -> zero row
        c_s = 0
        c_d = 0
        for t in range(blocks):
            kd = kmax_l[d][t]
            buf[:, c_d:c_d + kd] = idx_cols[d][:, c_s:c_s + kd]
            s = src_cols[d][:, c_s:c_s + kd]
            sbuf_[:, c_d:c_d + kd] = np.where(s < 0, cfg.N, s)
            c_s += kd
            c_d += kmax[t]
        idx_u.append(buf)
        msk_u.append(np.where(sbuf_ == cfg.N, MASK0, 0.0).astype(np.float32))
        # xg: per column block [128 feat, 128 dst] = x[src].T
        xgd = x_ext[sbuf_.T.reshape(-1)]           # [ncols*128, IN] bf16
        xgd = xgd.reshape(ncols, P, cfg.IN).transpose(0, 2, 1)  # [ncols, IN, P]
        xg_u.append(np.ascontiguousarray(
            xgd.transpose(1, 0, 2).reshape(cfg.IN, ncols * P)))

    # weights: A blockdiag for layer0 attention
    H, C = cfg.H, cfg.C
    A_src = np.zeros((H * C, H), np.float32)
    A_dst = np.zeros((H * C, H), np.float32)
    for h in range(H):
        A_src[h * C:(h + 1) * C, h] = att_src0[h]
        A_dst[h * C:(h + 1) * C, h] = att_dst0[h]
    w0cat = np.concatenate([W0, W0 @ A_src, W0 @ A_dst], axis=1)  # [IN, 80]
    w1cat = np.zeros((P, cfg.OUT + 2), np.float32)
    w1cat[:cfg.OUT] = np.concatenate(
        [W1, W1 @ att_src1[0][:, None], W1 @ att_dst1[0][:, None]], axis=1)
    biases = np.stack([b0.reshape(-1), b1.reshape(-1)])

    nc = _build(cfg, kmax, ncols)
    in_maps = []
    for d in range(NCORES):
        xt = np.zeros((cfg.IN, cfg.n_pad), ml_dtypes.bfloat16)
        xt[:, :cfg.n_per] = x_bf[perm_l[d]].T
        in_maps.append({
            "xt": xt,
            "xg": xg_u[d],
            "msk": msk_u[d],
            "idx": idx_u[d],
            "w0": w0cat.astype(ml_dtypes.bfloat16),
            "w1": w1cat,
            "bias": biases,
        })
    res = run_bass_kernel_spmd(nc, in_maps, core_ids=list(range(NCORES)))
    out = np.empty((cfg.N, cfg.OUT), np.float32)
    for d in range(NCORES):
        out[perm_l[d]] = res.results[d]["out"][:cfg.n_per]
    return out
